# revision 21
# baseline (speedup 1.0000x reference)
"""Neural CDE Trainium2 Bass kernel.

Problem: B=512, T=512, IN=6, H=64, HH=128, OUT=8.
RK4 scan over T-1=511 steps; vector field = 4-layer MLP + einsum with dX.
Sharding: data-parallel over batch, 64 batch elements per core, 8 cores.

Layout: feature-major activations (features on partitions, batch on free dim).
 - z_k^T = W^T z_{k-1}^T computed as matmul(lhsT=W, rhs=z^T)
 - biases: z1 via ones-row fold (K=65), z2/z3/tanh via ACT per-partition bias
 - einsum dh[b,h] = sum_i A[b,h,i] dX[b,i] done feature-major with
   host-permuted W_out (col order j = i*64+h):
     dXP[p, c, b] = dX^T[2c + (p>=64), b]  (selection matmuls, lhsT=E_c)
     U = A'_pre ... U[p,c,b] = A'^T[128c+p, b] * dXP[p,c,b]
     dh^T = S^T V  where V = sum_c U[:,c,:]  and S[p,h] = (p%64 == h)
 - RK4 combine accumulated in PSUM with scaled S matrices (S/6, S/3).
"""

import os
import numpy as np

B, T, IN, H, HH, OUT = 512, 512, 6, 64, 128, 8
NCORES = 8
BC = B // NCORES          # 64 batch per core
TS = int(os.environ.get("CDE_TS", str(T - 1)))   # RK4 steps (511)
TT = TS + 1               # number of output timesteps

F32 = None  # set lazily (mybir.dt.float32)
USE_Z1N = os.environ.get("CDE_Z1N", "1") == "1"

_BUILt = {}


def _build_bass():
    """Build the per-core Bass program (identical on all cores)."""
    import concourse.bass as bass
    import concourse.bacc as bacc
    import concourse.mybir as mybir
    from concourse.tile import TileContext

    f32 = mybir.dt.float32
    AF = mybir.ActivationFunctionType
    OP = mybir.AluOpType
    ds = bass.ds

    nc = bacc.Bacc("TRN2", debug=False, num_devices=NCORES)

    # ---- DRAM I/O ----
    d_x0T = nc.dram_tensor("x0T", [IN + 1, BC], f32, kind="ExternalInput")
    d_dXT = nc.dram_tensor("dXT", [IN, TS * BC], f32, kind="ExternalInput")
    d_winp = nc.dram_tensor("winp", [H + 1, HH], f32, kind="ExternalInput")
    d_wh1 = nc.dram_tensor("wh1", [HH, HH], f32, kind="ExternalInput")
    d_wh2 = nc.dram_tensor("wh2", [HH, HH], f32, kind="ExternalInput")
    d_bh = nc.dram_tensor("bh", [HH, 2], f32, kind="ExternalInput")
    d_woutp = nc.dram_tensor("woutp", [HH, 384], f32, kind="ExternalInput")
    d_boutp = nc.dram_tensor("boutp", [HH, 3], f32, kind="ExternalInput")
    d_wfinp = nc.dram_tensor("wfinp", [H + 1, OUT], f32, kind="ExternalInput")
    d_winitp = nc.dram_tensor("winitp", [IN + 1, H], f32, kind="ExternalInput")
    d_eall = nc.dram_tensor("eall", [IN, 384], f32, kind="ExternalInput")
    d_sall = nc.dram_tensor("sall", [HH, 128], f32, kind="ExternalInput")
    d_wstk = nc.dram_tensor("wstk", [HH, 512], f32, kind="ExternalInput")
    d_y = nc.dram_tensor("y", [BC, TT, OUT], f32, kind="ExternalOutput")

    # ---- persistent SBUF ----
    sb_dXT = nc.alloc_sbuf_tensor("sb_dXT", [IN, TS * BC], f32).ap()
    sb_x0T = nc.alloc_sbuf_tensor("sb_x0T", [IN + 1, BC], f32).ap()
    sb_winp = nc.alloc_sbuf_tensor("sb_winp", [H + 1, HH], f32).ap()
    sb_wh1 = nc.alloc_sbuf_tensor("sb_wh1", [HH, HH], f32).ap()
    sb_wh2 = nc.alloc_sbuf_tensor("sb_wh2", [HH, HH], f32).ap()
    sb_bh = nc.alloc_sbuf_tensor("sb_bh", [HH, 2], f32).ap()
    sb_woutp = nc.alloc_sbuf_tensor("sb_woutp", [HH, 384], f32).ap()
    sb_boutp = nc.alloc_sbuf_tensor("sb_boutp", [HH, 3], f32).ap()
    sb_wfinp = nc.alloc_sbuf_tensor("sb_wfinp", [H + 1, OUT], f32).ap()
    sb_winitp = nc.alloc_sbuf_tensor("sb_winitp", [IN + 1, H], f32).ap()
    sb_eall = nc.alloc_sbuf_tensor("sb_eall", [IN, 384], f32).ap()
    sb_sall = nc.alloc_sbuf_tensor("sb_sall", [HH, 128], f32).ap()
    sb_wstk = nc.alloc_sbuf_tensor("sb_wstk", [HH, 512], f32).ap()
    hT = nc.alloc_sbuf_tensor("hT", [H + 1, BC], f32).ap()
    ybuf = nc.alloc_sbuf_tensor("ybuf", [BC, TT * OUT], f32).ap()
    z1nA = nc.alloc_psum_tensor("z1nA", [HH, BC], f32).ap()
    z1nB = nc.alloc_psum_tensor("z1nB", [HH, BC], f32).ap()

    dXT3 = sb_dXT.rearrange("p (t b) -> p t b", b=BC)

    with TileContext(nc) as tc:
        with (
            tc.tile_pool(name="sb", bufs=3) as sbp,
            tc.tile_pool(name="ps", bufs=2, space="PSUM") as psp,
            tc.tile_pool(name="ps1", bufs=1, space="PSUM") as psp1,
        ):
            # ---- load everything ----
            for dst, src in [
                (sb_dXT, d_dXT.ap()), (sb_x0T, d_x0T.ap()),
                (sb_winp, d_winp.ap()), (sb_wh1, d_wh1.ap()),
                (sb_wh2, d_wh2.ap()), (sb_bh, d_bh.ap()),
                (sb_woutp, d_woutp.ap()), (sb_boutp, d_boutp.ap()),
                (sb_wfinp, d_wfinp.ap()), (sb_winitp, d_winitp.ap()),
                (sb_eall, d_eall.ap()), (sb_sall, d_sall.ap()),
                (sb_wstk, d_wstk.ap()),
            ]:
                nc.sync.dma_start(dst, src)

            # h0 = W_init^T x0^T + b_init  (feature-major)
            h0p = psp1.tile([H, BC], f32, tag="dxp")
            nc.tensor.matmul(h0p[:], sb_winitp, sb_x0T, start=True, stop=True)
            nc.scalar.copy(hT[0:H, :], h0p[:])

            # ones row for z1/y bias folding — on ACT so pre-loop matmuls
            # only wait on {ACT, DMA} (codegen max 2 sync waits per inst)
            nc.scalar.activation(
                hT[H:H + 1, :], h0p[0:1, :], AF.Copy, bias=1.0, scale=0.0)

            # y_0
            y0p = psp1.tile([BC, OUT], f32, tag="dxp")
            nc.tensor.matmul(y0p[:], hT, sb_wfinp, start=True, stop=True)
            nc.vector.tensor_copy(ybuf[:, 0:OUT], y0p[:])

            S_6 = sb_sall[:, 0:64]      # S/6
            S_3 = sb_sall[:, 64:128]    # S/3
            W_sh = sb_wstk[:, 0:128]    # 0.5*[W_in;W_in]
            W_sf = sb_wstk[:, 128:256]  # [W_in;W_in]
            W_s6 = sb_wstk[:, 256:384]  # [W_in;W_in]/6
            W_s3 = sb_wstk[:, 384:512]  # [W_in;W_in]/3

            def step_body(t, z1n_in, par):
                """One RK4 step. z1n_in: psum AP holding z1_pre(h_t)
                (base + U-corrections accumulated last step). Returns the
                z1n psum AP for step t+1 (bank alternates by parity)."""
                dxt = dXT3[:, ds(t, 1), :]                   # [6,1,64]
                dXPp = psp1.tile([HH, 192], f32, tag="dxp")  # [128,3,64]
                for c in range(3):
                    nc.tensor.matmul(
                        dXPp[:, 64 * c:64 * (c + 1)],
                        sb_eall[:, 128 * c:128 * (c + 1)], dxt,
                        start=True, stop=True)
                # keep dX broadcast in SBUF so the chain mults run SBUF-SBUF
                dXPs = sbp.tile([HH, 192], f32, tag="dxs")
                nc.scalar.copy(dXPs[:], dXPp[:])
                hacc = psp1.tile([H, BC], f32, tag="hacc")
                if USE_Z1N:
                    # output bank is opposite of input bank (par = t%2)
                    z1n = z1nA if par else z1nB
                    nc.tensor.matmul(z1n, sb_winp, hT, start=True, stop=False,
                                     skip_group_check=True)
                else:
                    z1n = None

                zps = []  # z1p psum tiles, one per vf
                if USE_Z1N:
                    zps.append(z1n_in)
                else:
                    zp_ = psp.tile([HH, BC], f32, tag="zp")
                    nc.tensor.matmul(zp_[:], sb_winp, hT,
                                     start=True, stop=True,
                                     skip_group_check=True)
                    zps.append(zp_)

                for v in range(4):
                    z1p = zps[v]
                    zs1 = sbp.tile([HH, BC], f32, tag="zs1")
                    nc.vector.tensor_scalar(zs1[:], z1p if v == 0 and USE_Z1N
                                            else z1p[:], 0.0, None, OP.max)
                    z2p = psp.tile([HH, BC], f32, tag="zp")
                    nc.tensor.matmul(z2p[:], sb_wh1, zs1[:], start=True, stop=True)
                    zs2 = sbp.tile([HH, BC], f32, tag="zs2")
                    nc.vector.tensor_scalar(zs2[:], z2p[:], sb_bh[:, 0:1], 0.0,
                                            OP.add, OP.max)
                    z3p = psp.tile([HH, BC], f32, tag="zp")
                    nc.tensor.matmul(z3p[:], sb_wh2, zs2[:], start=True, stop=True)
                    zs3 = sbp.tile([HH, BC], f32, tag="zs3")
                    nc.vector.tensor_scalar(zs3[:], z3p[:], sb_bh[:, 1:2], 0.0,
                                            OP.add, OP.max)
                    # A chunks pipelined: matmul -> tanh -> mult per chunk
                    Us = sbp.tile([HH, 192], f32, tag="us")
                    for c in range(3):
                        cs = slice(64 * c, 64 * (c + 1))
                        Ap = psp.tile([HH, BC], f32, tag="ap")
                        nc.tensor.matmul(Ap[:], sb_woutp[:, 128 * c:128 * (c + 1)],
                                         zs3[:], start=True, stop=True)
                        As = sbp.tile([HH, BC], f32, tag="as")
                        nc.scalar.activation(As[:], Ap[:], AF.Tanh,
                                             bias=sb_boutp[:, c:c + 1])
                        nc.vector.tensor_tensor(Us[:, cs], As[:], dXPs[:, cs],
                                                OP.mult)
                    # CRITICAL PATH FIRST: next vf's z1_pre correction
                    if v < 3:
                        zn_ = psp.tile([HH, BC], f32, tag="zp")
                        nc.tensor.matmul(zn_[:], sb_winp, hT,
                                         start=True, stop=False,
                                         skip_group_check=True)
                        wsl = W_sh if v < 2 else W_sf
                        for c in range(3):
                            nc.tensor.matmul(
                                zn_[:], wsl, Us[:, 64 * c:64 * (c + 1)],
                                start=False, stop=(c == 2),
                                skip_group_check=True)
                        zps.append(zn_)
                    # off-chain folds: z1n correction + RK4 combine
                    if USE_Z1N:
                        wnl = W_s6 if v in (0, 3) else W_s3
                        for c in range(3):
                            nc.tensor.matmul(z1n, wnl,
                                             Us[:, 64 * c:64 * (c + 1)],
                                             start=False,
                                             stop=(v == 3 and c == 2),
                                             skip_group_check=True)
                    sc = S_6 if v in (0, 3) else S_3
                    for c in range(3):
                        nc.tensor.matmul(hacc[:], sc,
                                         Us[:, 64 * c:64 * (c + 1)],
                                         start=(v == 0 and c == 0),
                                         stop=(v == 3 and c == 2),
                                         skip_group_check=True)
                # h_{t+1} = h_t + hacc  (off critical path)
                nc.vector.tensor_tensor(hT[0:H, :], hT[0:H, :], hacc[:], OP.add)
                # y_{t+1}
                yp = psp1.tile([BC, OUT], f32, tag="dxp")
                nc.tensor.matmul(yp[:], hT, sb_wfinp, start=True, stop=True)
                nc.vector.tensor_copy(ybuf[:, ds(t * OUT + OUT, OUT)], yp[:])
                return z1n

            # initial z1n = z1_pre(h_0) (base only)
            z1n = z1nA
            nc.tensor.matmul(z1n, sb_winp, hT, start=True, stop=True,
                             skip_group_check=True)

            unroll = int(os.environ.get("CDE_UNROLL", "4"))
            if unroll % 2:
                unroll *= 2  # parity of z1n banks must be loop-invariant
            n_loop = (TS // unroll) * unroll
            if n_loop:
                with tc.For_i(0, n_loop, unroll) as i:
                    for u in range(unroll):
                        z1n = step_body(i + u, z1n, u % 2)
            for t in range(n_loop, TS):
                z1n = step_body(t, z1n, (t - n_loop) % 2)

            # write out
            nc.sync.dma_start(d_y.ap().rearrange("b t o -> b (t o)"), ybuf)

    nc.compile()
    return nc


def _host_prep(x, W_init, b_init, W_in, b_in, W_h1, b_h1, W_h2, b_h2,
               W_out, b_out, W_fin, b_fin):
    """Build shared weight tensors + per-core input shards (numpy, fp32)."""
    f32 = np.float32
    winp = np.concatenate([W_in, b_in[None, :]], 0).astype(f32)          # [65,128]
    bh = np.stack([b_h1, b_h2], 1).astype(f32)                           # [128,2]
    # permute W_out columns: j_new = i*64 + h  (from j_old = h*6 + i)
    woutp = np.ascontiguousarray(
        W_out.reshape(HH, H, IN).transpose(0, 2, 1).reshape(HH, H * IN)).astype(f32)
    boutp_flat = np.ascontiguousarray(
        b_out.reshape(H, IN).T.reshape(H * IN)).astype(f32)              # [384]
    boutp = boutp_flat.reshape(3, HH).T.copy()                           # [128,3]
    wfinp = np.concatenate([W_fin, b_fin[None, :]], 0).astype(f32)       # [65,8]
    winitp = np.concatenate([W_init, b_init[None, :]], 0).astype(f32)    # [7,64]
    # E_all[i, 128c + p] = 1 if i == 2c + (p>=64)
    eall = np.zeros((IN, 384), f32)
    for c in range(3):
        eall[2 * c, 128 * c:128 * c + 64] = 1.0
        eall[2 * c + 1, 128 * c + 64:128 * c + 128] = 1.0
    # S[p, h] = (p % 64 == h); scaled variants
    S = np.zeros((HH, H), f32)
    for p in range(HH):
        S[p, p % H] = 1.0
    sall = np.concatenate([S / 6.0, S / 3.0], 1).astype(f32)             # [128,128]
    stk = np.concatenate([W_in, W_in], 0).astype(f32)                    # [128,128]
    wstk = np.concatenate([0.5 * stk, stk, stk / 6.0, stk / 3.0], 1).astype(f32)

    shared = dict(winp=winp, wh1=np.ascontiguousarray(W_h1, f32),
                  wh2=np.ascontiguousarray(W_h2, f32), bh=bh,
                  woutp=woutp, boutp=boutp, wfinp=wfinp, winitp=winitp,
                  eall=eall, sall=sall, wstk=wstk)

    in_maps = []
    for c in range(NCORES):
        xs = np.asarray(x[c * BC:(c + 1) * BC], f32)                     # [64,512,6]
        x0T = np.concatenate([xs[:, 0, :].T, np.ones((1, BC), f32)], 0)  # [7,64]
        dX = (xs[:, 1:, :] - xs[:, :-1, :])[:, :TS, :]                   # [64,TS,6]
        dXT = np.ascontiguousarray(dX.transpose(2, 1, 0)).reshape(IN, TS * BC)
        m = dict(shared)
        m["x0T"] = np.ascontiguousarray(x0T)
        m["dXT"] = dXT
        in_maps.append(m)
    return in_maps


def kernel(**inputs):
    from concourse.bass_utils import run_bass_kernel_spmd

    np_inputs = {k: np.asarray(v, np.float32) for k, v in inputs.items()}
    in_maps = _host_prep(**np_inputs)

    if "nc" not in _BUILt:
        _BUILt["nc"] = _build_bass()
    nc = _BUILt["nc"]

    trace = os.environ.get("CDE_TRACE", "0") == "1"
    res = run_bass_kernel_spmd(
        nc, in_maps, core_ids=list(range(NCORES)), trace=trace)
    if trace and res.exec_time_ns is not None:
        print(f"HW exec time: {res.exec_time_ns} ns")
        _BUILt["exec_time_ns"] = res.exec_time_ns
    y = np.concatenate([r["y"] for r in res.results], 0)   # [512,512,8]
    return y


# revision 22
# speedup vs baseline: 4.6414x; 4.6414x over previous
"""Neural CDE Trainium2 Bass kernel.

Problem: B=512, T=512, IN=6, H=64, HH=128, OUT=8.
RK4 scan over T-1=511 steps; vector field = 4-layer MLP + einsum with dX.
Sharding: data-parallel over batch, 64 batch elements per core, 8 cores.

Layout: feature-major activations (features on partitions, batch on free dim).
 - z_k^T = W^T z_{k-1}^T computed as matmul(lhsT=W, rhs=z^T)
 - biases: z1 via ones-row fold (K=65), z2/z3/tanh via ACT per-partition bias
 - einsum dh[b,h] = sum_i A[b,h,i] dX[b,i] done feature-major with
   host-permuted W_out (col order j = i*64+h):
     dXP[p, c, b] = dX^T[2c + (p>=64), b]  (selection matmuls, lhsT=E_c)
     U = A'_pre ... U[p,c,b] = A'^T[128c+p, b] * dXP[p,c,b]
     dh^T = S^T V  where V = sum_c U[:,c,:]  and S[p,h] = (p%64 == h)
 - RK4 combine accumulated in PSUM with scaled S matrices (S/6, S/3).
"""

import os
import numpy as np

B, T, IN, H, HH, OUT = 512, 512, 6, 64, 128, 8
NCORES = 8
BC = B // NCORES          # 64 batch per core
TS = int(os.environ.get("CDE_TS", str(T - 1)))   # RK4 steps (511)
TT = TS + 1               # number of output timesteps

F32 = None  # set lazily (mybir.dt.float32)
USE_Z1N = os.environ.get("CDE_Z1N", "1") == "1"

_BUILt = {}


def _build_bass():
    """Build the per-core Bass program (identical on all cores)."""
    import concourse.bass as bass
    import concourse.bacc as bacc
    import concourse.mybir as mybir
    from concourse.tile import TileContext

    f32 = mybir.dt.float32
    AF = mybir.ActivationFunctionType
    OP = mybir.AluOpType
    ds = bass.ds

    nc = bacc.Bacc("TRN2", debug=False, num_devices=NCORES)

    # ---- DRAM I/O ----
    d_x0T = nc.dram_tensor("x0T", [IN + 1, BC], f32, kind="ExternalInput")
    d_dXT = nc.dram_tensor("dXT", [IN, TS * BC], f32, kind="ExternalInput")
    d_winp = nc.dram_tensor("winp", [H + 1, HH], f32, kind="ExternalInput")
    d_wh1 = nc.dram_tensor("wh1", [HH, HH], f32, kind="ExternalInput")
    d_wh2 = nc.dram_tensor("wh2", [HH, HH], f32, kind="ExternalInput")
    d_bh = nc.dram_tensor("bh", [HH, 2], f32, kind="ExternalInput")
    d_woutp = nc.dram_tensor("woutp", [HH, 384], f32, kind="ExternalInput")
    d_boutp = nc.dram_tensor("boutp", [HH, 3], f32, kind="ExternalInput")
    d_wfinp = nc.dram_tensor("wfinp", [H + 1, OUT], f32, kind="ExternalInput")
    d_winitp = nc.dram_tensor("winitp", [IN + 1, H], f32, kind="ExternalInput")
    d_eall = nc.dram_tensor("eall", [IN, 384], f32, kind="ExternalInput")
    d_sall = nc.dram_tensor("sall", [HH, 128], f32, kind="ExternalInput")
    d_wstk = nc.dram_tensor("wstk", [HH, 512], f32, kind="ExternalInput")
    d_y = nc.dram_tensor("y", [BC, TT, OUT], f32, kind="ExternalOutput")

    # ---- persistent SBUF ----
    sb_dXT = nc.alloc_sbuf_tensor("sb_dXT", [IN, TS * BC], f32).ap()
    sb_x0T = nc.alloc_sbuf_tensor("sb_x0T", [IN + 1, BC], f32).ap()
    sb_winp = nc.alloc_sbuf_tensor("sb_winp", [H + 1, HH], f32).ap()
    sb_wh1 = nc.alloc_sbuf_tensor("sb_wh1", [HH, HH], f32).ap()
    sb_wh2 = nc.alloc_sbuf_tensor("sb_wh2", [HH, HH], f32).ap()
    sb_bh = nc.alloc_sbuf_tensor("sb_bh", [HH, 2], f32).ap()
    sb_woutp = nc.alloc_sbuf_tensor("sb_woutp", [HH, 384], f32).ap()
    sb_boutp = nc.alloc_sbuf_tensor("sb_boutp", [HH, 3], f32).ap()
    sb_wfinp = nc.alloc_sbuf_tensor("sb_wfinp", [H + 1, OUT], f32).ap()
    sb_winitp = nc.alloc_sbuf_tensor("sb_winitp", [IN + 1, H], f32).ap()
    sb_eall = nc.alloc_sbuf_tensor("sb_eall", [IN, 384], f32).ap()
    sb_sall = nc.alloc_sbuf_tensor("sb_sall", [HH, 128], f32).ap()
    sb_wstk = nc.alloc_sbuf_tensor("sb_wstk", [HH, 512], f32).ap()
    hT = nc.alloc_sbuf_tensor("hT", [H + 1, BC], f32).ap()
    ybuf = nc.alloc_sbuf_tensor("ybuf", [BC, TT * OUT], f32).ap()
    z1nA = nc.alloc_psum_tensor("z1nA", [HH, BC], f32).ap()
    z1nB = nc.alloc_psum_tensor("z1nB", [HH, BC], f32).ap()

    dXT3 = sb_dXT.rearrange("p (t b) -> p t b", b=BC)

    with TileContext(nc) as tc:
        with (
            tc.tile_pool(name="sb", bufs=3) as sbp,
            tc.tile_pool(name="ps", bufs=2, space="PSUM") as psp,
            tc.tile_pool(name="ps1", bufs=1, space="PSUM") as psp1,
        ):
            # ---- load everything ----
            for dst, src in [
                (sb_dXT, d_dXT.ap()), (sb_x0T, d_x0T.ap()),
                (sb_winp, d_winp.ap()), (sb_wh1, d_wh1.ap()),
                (sb_wh2, d_wh2.ap()), (sb_bh, d_bh.ap()),
                (sb_woutp, d_woutp.ap()), (sb_boutp, d_boutp.ap()),
                (sb_wfinp, d_wfinp.ap()), (sb_winitp, d_winitp.ap()),
                (sb_eall, d_eall.ap()), (sb_sall, d_sall.ap()),
                (sb_wstk, d_wstk.ap()),
            ]:
                nc.sync.dma_start(dst, src)

            # h0 = W_init^T x0^T + b_init  (feature-major)
            h0p = psp1.tile([H, BC], f32, tag="dxp")
            nc.tensor.matmul(h0p[:], sb_winitp, sb_x0T, start=True, stop=True)
            nc.scalar.copy(hT[0:H, :], h0p[:])

            # ones row for z1/y bias folding — on ACT so pre-loop matmuls
            # only wait on {ACT, DMA} (codegen max 2 sync waits per inst)
            nc.scalar.activation(
                hT[H:H + 1, :], h0p[0:1, :], AF.Copy, bias=1.0, scale=0.0)

            # y_0
            y0p = psp1.tile([BC, OUT], f32, tag="dxp")
            nc.tensor.matmul(y0p[:], hT, sb_wfinp, start=True, stop=True)
            nc.vector.tensor_copy(ybuf[:, 0:OUT], y0p[:])

            S_6 = sb_sall[:, 0:64]      # S/6
            S_3 = sb_sall[:, 64:128]    # S/3
            W_sh = sb_wstk[:, 0:128]    # 0.5*[W_in;W_in]
            W_sf = sb_wstk[:, 128:256]  # [W_in;W_in]
            W_s6 = sb_wstk[:, 256:384]  # [W_in;W_in]/6
            W_s3 = sb_wstk[:, 384:512]  # [W_in;W_in]/3

            def step_body(t, z1n_in, par):
                """One RK4 step. z1n_in: psum AP holding z1_pre(h_t)
                (base + U-corrections accumulated last step). Returns the
                z1n psum AP for step t+1 (bank alternates by parity)."""
                dxt = dXT3[:, ds(t, 1), :]                   # [6,1,64]
                dXPp = psp1.tile([HH, 192], f32, tag="dxp")  # [128,3,64]
                for c in range(3):
                    nc.tensor.matmul(
                        dXPp[:, 64 * c:64 * (c + 1)],
                        sb_eall[:, 128 * c:128 * (c + 1)], dxt,
                        start=True, stop=True)
                # keep dX broadcast in SBUF so the chain mults run SBUF-SBUF
                dXPs = sbp.tile([HH, 192], f32, tag="dxs")
                nc.scalar.copy(dXPs[:], dXPp[:])
                hacc = psp1.tile([H, BC], f32, tag="hacc")
                if USE_Z1N:
                    # output bank is opposite of input bank (par = t%2)
                    z1n = z1nA if par else z1nB
                    nc.tensor.matmul(z1n, sb_winp, hT, start=True, stop=False,
                                     skip_group_check=True)
                else:
                    z1n = None

                zps = []  # z1p psum tiles, one per vf
                if USE_Z1N:
                    zps.append(z1n_in)
                else:
                    zp_ = psp.tile([HH, BC], f32, tag="zp")
                    nc.tensor.matmul(zp_[:], sb_winp, hT,
                                     start=True, stop=True,
                                     skip_group_check=True)
                    zps.append(zp_)

                for v in range(4):
                    z1p = zps[v]
                    zs1 = sbp.tile([HH, BC], f32, tag="zs1")
                    nc.vector.tensor_scalar(zs1[:], z1p if v == 0 and USE_Z1N
                                            else z1p[:], 0.0, None, OP.max)
                    z2p = psp.tile([HH, BC], f32, tag="zp")
                    nc.tensor.matmul(z2p[:], sb_wh1, zs1[:], start=True, stop=True)
                    zs2 = sbp.tile([HH, BC], f32, tag="zs2")
                    nc.vector.tensor_scalar(zs2[:], z2p[:], sb_bh[:, 0:1], 0.0,
                                            OP.add, OP.max)
                    z3p = psp.tile([HH, BC], f32, tag="zp")
                    nc.tensor.matmul(z3p[:], sb_wh2, zs2[:], start=True, stop=True)
                    zs3 = sbp.tile([HH, BC], f32, tag="zs3")
                    nc.vector.tensor_scalar(zs3[:], z3p[:], sb_bh[:, 1:2], 0.0,
                                            OP.add, OP.max)
                    # A chunks pipelined: matmul -> tanh -> mult per chunk
                    Us = sbp.tile([HH, 192], f32, tag="us")
                    for c in range(3):
                        cs = slice(64 * c, 64 * (c + 1))
                        Ap = psp.tile([HH, BC], f32, tag="ap")
                        nc.tensor.matmul(Ap[:], sb_woutp[:, 128 * c:128 * (c + 1)],
                                         zs3[:], start=True, stop=True)
                        As = sbp.tile([HH, BC], f32, tag="as")
                        nc.scalar.activation(As[:], Ap[:], AF.Tanh,
                                             bias=sb_boutp[:, c:c + 1])
                        nc.vector.tensor_tensor(Us[:, cs], As[:], dXPs[:, cs],
                                                OP.mult)
                    # CRITICAL PATH FIRST: next vf's z1_pre correction
                    if v < 3:
                        zn_ = psp.tile([HH, BC], f32, tag="zp")
                        nc.tensor.matmul(zn_[:], sb_winp, hT,
                                         start=True, stop=False,
                                         skip_group_check=True)
                        wsl = W_sh if v < 2 else W_sf
                        for c in range(3):
                            nc.tensor.matmul(
                                zn_[:], wsl, Us[:, 64 * c:64 * (c + 1)],
                                start=False, stop=(c == 2),
                                skip_group_check=True)
                        zps.append(zn_)
                    # off-chain folds: z1n correction + RK4 combine
                    if USE_Z1N:
                        wnl = W_s6 if v in (0, 3) else W_s3
                        for c in range(3):
                            nc.tensor.matmul(z1n, wnl,
                                             Us[:, 64 * c:64 * (c + 1)],
                                             start=False,
                                             stop=(v == 3 and c == 2),
                                             skip_group_check=True)
                    sc = S_6 if v in (0, 3) else S_3
                    for c in range(3):
                        nc.tensor.matmul(hacc[:], sc,
                                         Us[:, 64 * c:64 * (c + 1)],
                                         start=(v == 0 and c == 0),
                                         stop=(v == 3 and c == 2),
                                         skip_group_check=True)
                # h_{t+1} = h_t + hacc  (off critical path)
                nc.vector.tensor_tensor(hT[0:H, :], hT[0:H, :], hacc[:], OP.add)
                # y_{t+1}
                yp = psp1.tile([BC, OUT], f32, tag="dxp")
                nc.tensor.matmul(yp[:], hT, sb_wfinp, start=True, stop=True)
                nc.vector.tensor_copy(ybuf[:, ds(t * OUT + OUT, OUT)], yp[:])
                return z1n

            # initial z1n = z1_pre(h_0) (base only)
            z1n = z1nA
            nc.tensor.matmul(z1n, sb_winp, hT, start=True, stop=True,
                             skip_group_check=True)

            unroll = int(os.environ.get("CDE_UNROLL", "4"))
            if unroll % 2:
                unroll *= 2  # parity of z1n banks must be loop-invariant
            n_loop = (TS // unroll) * unroll
            repeat = int(os.environ.get("CDE_REPEAT", "1"))
            for _rep in range(repeat):
                if n_loop:
                    with tc.For_i(0, n_loop, unroll) as i:
                        for u in range(unroll):
                            z1n = step_body(i + u, z1n, u % 2)
                for t in range(n_loop, TS):
                    z1n = step_body(t, z1n, (t - n_loop) % 2)

            # write out
            nc.sync.dma_start(d_y.ap().rearrange("b t o -> b (t o)"), ybuf)

    nc.compile()
    return nc


def _host_prep(x, W_init, b_init, W_in, b_in, W_h1, b_h1, W_h2, b_h2,
               W_out, b_out, W_fin, b_fin):
    """Build shared weight tensors + per-core input shards (numpy, fp32)."""
    f32 = np.float32
    winp = np.concatenate([W_in, b_in[None, :]], 0).astype(f32)          # [65,128]
    bh = np.stack([b_h1, b_h2], 1).astype(f32)                           # [128,2]
    # permute W_out columns: j_new = i*64 + h  (from j_old = h*6 + i)
    woutp = np.ascontiguousarray(
        W_out.reshape(HH, H, IN).transpose(0, 2, 1).reshape(HH, H * IN)).astype(f32)
    boutp_flat = np.ascontiguousarray(
        b_out.reshape(H, IN).T.reshape(H * IN)).astype(f32)              # [384]
    boutp = boutp_flat.reshape(3, HH).T.copy()                           # [128,3]
    wfinp = np.concatenate([W_fin, b_fin[None, :]], 0).astype(f32)       # [65,8]
    winitp = np.concatenate([W_init, b_init[None, :]], 0).astype(f32)    # [7,64]
    # E_all[i, 128c + p] = 1 if i == 2c + (p>=64)
    eall = np.zeros((IN, 384), f32)
    for c in range(3):
        eall[2 * c, 128 * c:128 * c + 64] = 1.0
        eall[2 * c + 1, 128 * c + 64:128 * c + 128] = 1.0
    # S[p, h] = (p % 64 == h); scaled variants
    S = np.zeros((HH, H), f32)
    for p in range(HH):
        S[p, p % H] = 1.0
    sall = np.concatenate([S / 6.0, S / 3.0], 1).astype(f32)             # [128,128]
    stk = np.concatenate([W_in, W_in], 0).astype(f32)                    # [128,128]
    wstk = np.concatenate([0.5 * stk, stk, stk / 6.0, stk / 3.0], 1).astype(f32)

    shared = dict(winp=winp, wh1=np.ascontiguousarray(W_h1, f32),
                  wh2=np.ascontiguousarray(W_h2, f32), bh=bh,
                  woutp=woutp, boutp=boutp, wfinp=wfinp, winitp=winitp,
                  eall=eall, sall=sall, wstk=wstk)

    in_maps = []
    for c in range(NCORES):
        xs = np.asarray(x[c * BC:(c + 1) * BC], f32)                     # [64,512,6]
        x0T = np.concatenate([xs[:, 0, :].T, np.ones((1, BC), f32)], 0)  # [7,64]
        dX = (xs[:, 1:, :] - xs[:, :-1, :])[:, :TS, :]                   # [64,TS,6]
        dXT = np.ascontiguousarray(dX.transpose(2, 1, 0)).reshape(IN, TS * BC)
        m = dict(shared)
        m["x0T"] = np.ascontiguousarray(x0T)
        m["dXT"] = dXT
        in_maps.append(m)
    return in_maps


def kernel(**inputs):
    from concourse.bass_utils import run_bass_kernel_spmd

    np_inputs = {k: np.asarray(v, np.float32) for k, v in inputs.items()}
    in_maps = _host_prep(**np_inputs)

    if "nc" not in _BUILt:
        _BUILt["nc"] = _build_bass()
    nc = _BUILt["nc"]

    trace = os.environ.get("CDE_TRACE", "0") == "1"
    res = run_bass_kernel_spmd(
        nc, in_maps, core_ids=list(range(NCORES)), trace=trace)
    if trace and res.exec_time_ns is not None:
        print(f"HW exec time: {res.exec_time_ns} ns")
        _BUILt["exec_time_ns"] = res.exec_time_ns
    y = np.concatenate([r["y"] for r in res.results], 0)   # [512,512,8]
    return y
